# revision 29
# baseline (speedup 1.0000x reference)
"""Biased axial attention on 8 TRN2 NeuronCores (Bass/Tile SPMD kernel), v3.

Sharding: leading (non-attended) L axis n across 8 cores; bias rows i-sharded.
Pipeline: pair LN + projections (bf16) -> per-head QK partial logits ->
bf16 ReduceScatter overlapped with the bias-LN/blog phase -> sharded softmax
-> transpose shard -> bf16 AllGather of a^T -> AV with 128-wide packed
stationary -> tanh-gate -> output projection with row-group interleaved
accumulation chains.

Key scheduling ideas vs v2: bias-LN phase moved AFTER the QK+ReduceScatter
launch so the collectives hide under bias compute; all q/k/logits tensors in
bf16 (fp8 without DoubleRow has no PE advantage); DMA loads/stores batched
4 rows per descriptor; DVE work statically balanced across Vector/Scalar/
GpSimd with a stable ACT-table schedule (Sqrt in LN phases, one Tanh batch,
one Exp for softmax).
"""

import math
import numpy as np
import ml_dtypes

import concourse.bass as bass
import concourse.bacc as bacc
import concourse.tile as tile
from concourse import mybir
from concourse.bass_utils import run_bass_kernel_spmd

BF16 = mybir.dt.bfloat16
F32 = mybir.dt.float32
NPBF16 = ml_dtypes.bfloat16

LAST_RESULT = None

NCORES = 8
L = 384
DP = 128
DB = 128
H = 4
D = 32
HD = H * D
NLOC = L // NCORES  # 48
TC = 3
NG = NLOC // 4  # 12 groups of 4 rows
EPS = 1e-5
AF = mybir.ActivationFunctionType
ALU = mybir.AluOpType

GBF_HALF = 0.5  # gbf[0]*0.5, validated uniform in prepare()


def build_program():
    nc = bacc.Bacc(
        "TRN2",
        target_bir_lowering=False,
        debug=False,
        enable_asserts=False,
        num_devices=NCORES,
    )

    pair_s = nc.dram_tensor("pair_s", [NLOC, L, DP], BF16, kind="ExternalInput").ap()
    bias_s = nc.dram_tensor("bias_s", [NLOC, L, DB], BF16, kind="ExternalInput").ap()
    wq_d = nc.dram_tensor("wq", [DP, HD], BF16, kind="ExternalInput").ap()
    wk_d = nc.dram_tensor("wk", [DP, HD], BF16, kind="ExternalInput").ap()
    wv_d = nc.dram_tensor("wv", [DP, HD], BF16, kind="ExternalInput").ap()
    wg_d = nc.dram_tensor("wg", [DP, HD], BF16, kind="ExternalInput").ap()
    wb_d = nc.dram_tensor("wb", [DB, 32], BF16, kind="ExternalInput").ap()
    wout_d = nc.dram_tensor("wout", [128, H, DP], BF16, kind="ExternalInput").ap()
    # cvec[:,0]=eps, 1=gbf/2 (per hd)
    cvec_d = nc.dram_tensor("cvec", [128, 4], F32, kind="ExternalInput").ap()
    ident_d = nc.dram_tensor("ident", [128, 128], BF16, kind="ExternalInput").ap()
    out_d = nc.dram_tensor("out", [NLOC, L, DP], BF16, kind="ExternalOutput").ap()

    with tile.TileContext(nc) as tc, tc.tile_pool(name="persist", bufs=1) as pp:
        wq_sb = pp.tile([DP, HD], BF16, name="wq_sb")
        wk_sb = pp.tile([DP, HD], BF16, name="wk_sb")
        wv_sb = pp.tile([DP, HD], BF16, name="wv_sb")
        wg_sb = pp.tile([DP, HD], BF16, name="wg_sb")
        wb_sb = pp.tile([DB, 32], BF16, name="wb_sb")
        wout_sb = pp.tile([128, H, DP], BF16, name="wout_sb")
        cvec_sb = pp.tile([128, 4], F32, name="cvec_sb")
        ident_sb = pp.tile([128, 128], BF16, name="ident_sb")

        nc.gpsimd.dma_start(out=wq_sb[:], in_=wq_d[:])
        nc.gpsimd.dma_start(out=wk_sb[:], in_=wk_d[:])
        nc.gpsimd.dma_start(out=wv_sb[:], in_=wv_d[:])
        nc.gpsimd.dma_start(out=wg_sb[:], in_=wg_d[:])
        nc.gpsimd.dma_start(out=wb_sb[:], in_=wb_d[:])
        nc.gpsimd.dma_start(out=wout_sb[:], in_=wout_d[:])
        nc.gpsimd.dma_start(out=cvec_sb[:], in_=cvec_d[:])
        nc.sync.dma_start(out=ident_sb[:], in_=ident_d[:])

        # persistent operand buffers
        v_buf = pp.tile([128, TC, H, NLOC, D], BF16, name="v_buf")
        g_pk = pp.tile([128, NG, H, L], BF16, name="g_pk")
        blog_sb = pp.tile([NLOC, H, L], BF16, name="blog_sb")

        with tc.tile_pool(name="dram", bufs=1, space="DRAM") as dram:
            bounce = dram.tile([H, L, L], BF16)
            blog_d = dram.tile([NLOC, H, L], BF16)
            shard = dram.tile([H, NLOC, L], BF16)
            ags = dram.tile([H, TC, 128, NLOC], BF16)
            agouts = []
            for hh in range(H):
                agouts.append(dram.tile([NCORES, TC, 128, NLOC], BF16,
                                        addr_space="Shared",
                                        name=f"agout{hh}"))

            qk_pool_ctx = tc.tile_pool(name="qkpk", bufs=1)
            qkp = qk_pool_ctx.__enter__()
            qkpk = qkp.tile([128, NG, H, 2, L], BF16, name="qkpk")

            # =========================================================
            # Phase 1: pair LN + transposes + projections
            # =========================================================
            with (
                tc.tile_pool(name="xt", bufs=3) as xt_pool,
                tc.tile_pool(name="bx", bufs=3) as bx_pool,
                tc.tile_pool(name="st", bufs=4) as st_pool,
                tc.tile_pool(name="fx", bufs=6) as fx_pool,
                tc.tile_pool(name="xn", bufs=6) as xn_pool,
                tc.tile_pool(name="xnt", bufs=8) as xnt_pool,
                tc.tile_pool(name="blg", bufs=2) as blg_pool,
                tc.tile_pool(name="lg", bufs=3) as lg_pool,
                tc.tile_pool(name="psT", bufs=2, space="PSUM") as psT_pool,
                tc.tile_pool(name="psQK", bufs=2, space="PSUM") as psQK_pool,
                tc.tile_pool(name="psP", bufs=2, space="PSUM") as psP_pool,
            ):

                def stats_group(xts, fx):
                    """bn_stats + fixups for 4 rows; writes fx[:, 0:12]=rstd,
                    fx[:, 12:24]=nmr=(-mean*rstd), f32. Col 3*ns+tc."""
                    st = st_pool.tile([128, 4, TC, 6], F32, name="st")
                    for ns in range(4):
                        for tcc in range(TC):
                            nc.vector.bn_stats(out=st[:, ns, tcc],
                                               in_=xts[ns][:, tcc])
                    w = st_pool.tile([128, 4, TC, 4], F32, name="w")
                    sv = st.rearrange("p a b c -> p (a b) c")
                    wv_ = w.rearrange("p a b c -> p (a b) c")
                    # w0 = 0.5*(me - mo); w1 = me + mo (= 2*mean)
                    nc.vector.tensor_tensor(
                        out=wv_[:, :, 0], in0=sv[:, :, 1], in1=sv[:, :, 4],
                        op=ALU.subtract)
                    nc.vector.tensor_scalar(
                        out=wv_[:, :, 0], in0=wv_[:, :, 0], scalar1=0.5,
                        scalar2=None, op0=ALU.mult)
                    nc.vector.tensor_tensor(
                        out=wv_[:, :, 1], in0=sv[:, :, 1], in1=sv[:, :, 4],
                        op=ALU.add)
                    # w2 = (M2e + M2o)/128 + w0^2  (= var)
                    nc.vector.tensor_tensor(
                        out=wv_[:, :, 2], in0=sv[:, :, 2], in1=sv[:, :, 5],
                        op=ALU.add)
                    nc.vector.tensor_tensor(
                        out=wv_[:, :, 0], in0=wv_[:, :, 0], in1=wv_[:, :, 0],
                        op=ALU.mult)
                    nc.vector.scalar_tensor_tensor(
                        out=wv_[:, :, 2], in0=wv_[:, :, 2], scalar=1.0 / 128.0,
                        in1=wv_[:, :, 0], op0=ALU.mult, op1=ALU.add)
                    # sd = sqrt(var + eps); rstd = 1/sd; nmr = -mean*rstd
                    nc.scalar.activation(
                        out=wv_[:, :, 3], in_=wv_[:, :, 2], func=AF.Sqrt,
                        bias=cvec_sb[:, 0:1], scale=1.0)
                    nc.vector.reciprocal(out=fx[:, 0:12], in_=wv_[:, :, 3])
                    nc.vector.scalar_tensor_tensor(
                        out=fx[:, 12:24], in0=wv_[:, :, 1], scalar=-0.5,
                        in1=fx[:, 0:12], op0=ALU.mult, op1=ALU.mult)

                def normalize(xts, fx, ns, eng):
                    """xn = (x + nmr)*rstd per chunk, bf16."""
                    xn = xn_pool.tile([128, TC, 128], BF16, name="xn")
                    for tcc in range(TC):
                        c = 3 * ns + tcc
                        if eng == "act":
                            nc.scalar.activation(
                                out=xn[:, tcc], in_=xts[ns][:, tcc],
                                func=AF.Identity,
                                bias=fx[:, 12 + c:13 + c],
                                scale=fx[:, c:c + 1])
                        else:
                            e = nc.gpsimd if eng == "pool" else nc.vector
                            e.tensor_scalar(
                                out=xn[:, tcc], in0=xts[ns][:, tcc],
                                scalar1=fx[:, 12 + c:13 + c],
                                scalar2=fx[:, c:c + 1],
                                op0=ALU.add, op1=ALU.mult)
                    return xn

                for g in range(NG):
                    # batched load: 4 rows in one DMA
                    xt4 = xt_pool.tile([128, 4, TC, DP], BF16, name="xt4")
                    nc.sync.dma_start(
                        out=xt4[:],
                        in_=pair_s[4 * g:4 * g + 4].rearrange(
                            "n (c p) d -> p n c d", p=128))
                    xts = [xt4[:, ns] for ns in range(4)]
                    fx = fx_pool.tile([128, 24], F32, name="fxp")
                    stats_group(xts, fx)
                    xnts = []
                    for np_ in range(2):
                        pst = psT_pool.tile([128, 2, L], BF16, name="pst",
                                            tag="pst")
                        for sub in range(2):
                            ns = 2 * np_ + sub
                            xn = normalize(xts, fx, ns, "pool")
                            for tcc in range(TC):
                                nc.tensor.transpose(
                                    out=pst[:, sub, tcc * 128:(tcc + 1) * 128],
                                    in_=xn[:, tcc], identity=ident_sb[:])
                        xnt2 = xnt_pool.tile([128, 2, L], BF16, name="xnt")
                        xnts.append(xnt2[:, 0])
                        xnts.append(xnt2[:, 1])
                        # bf16 PSUM -> SBUF drain on Vector (2x mode)
                        nc.vector.tensor_copy(out=xnt2[:], in_=pst[:])
                        for sub in range(2):
                            ns = 2 * np_ + sub
                            n = 4 * g + ns
                            # v projection, token-major
                            psv = psP_pool.tile([128, TC, DP], F32, name="psv",
                                                tag="pj")
                            for tcc in range(TC):
                                nc.tensor.matmul(
                                    out=psv[:, tcc],
                                    lhsT=xnt2[:, sub,
                                              tcc * 128:(tcc + 1) * 128],
                                    rhs=wv_sb[:], start=True, stop=True)
                            nc.scalar.activation(
                                out=v_buf[:, :, :, n, :],
                                in_=psv.rearrange("p a (h d) -> p a h d",
                                                  h=H),
                                func=AF.Copy, bias=0.0, scale=1.0)
                    # packed q/k/g projections: [(ns,d), i] per (g, h)
                    for h in range(H):
                        psqk = psQK_pool.tile([128, 2, 512], F32, name="psqk",
                                              tag="pqk")
                        for ns in range(4):
                            nc.tensor.matmul(
                                out=psqk[32 * ns:32 * ns + 32, 0, 0:L],
                                lhsT=wq_sb[:, 32 * h:32 * h + 32],
                                rhs=xnts[ns][:], start=True, stop=True,
                                tile_position=(0, 32 * ns))
                        for ns in range(4):
                            nc.tensor.matmul(
                                out=psqk[32 * ns:32 * ns + 32, 1, 0:L],
                                lhsT=wk_sb[:, 32 * h:32 * h + 32],
                                rhs=xnts[ns][:], start=True, stop=True,
                                tile_position=(0, 32 * ns))
                        if h % 2 == 0:
                            nc.scalar.activation(out=qkpk[:, g, h, :, :],
                                                 in_=psqk[:, :, 0:L],
                                                 func=AF.Copy, bias=0.0,
                                                 scale=1.0)
                        else:
                            nc.vector.tensor_copy(out=qkpk[:, g, h, :, :],
                                                  in_=psqk[:, :, 0:L])
                        psg = psP_pool.tile([128, L], F32, name="ps", tag="pj")
                        for ns in range(4):
                            nc.tensor.matmul(
                                out=psg[32 * ns:32 * ns + 32, :],
                                lhsT=wg_sb[:, 32 * h:32 * h + 32],
                                rhs=xnts[ns][:], start=True, stop=True,
                                tile_position=(0, 32 * ns))
                        nc.vector.tensor_copy(out=g_pk[:, g, h, :],
                                              in_=psg[:])

                # tanh(g/2 + gbf/2) in place (one op, one table load)
                gflat = g_pk.rearrange("p a b c -> p (a b c)")
                nc.scalar.activation(
                    out=gflat[:], in_=gflat[:],
                    func=AF.Tanh, bias=cvec_sb[:, 1:2], scale=0.5)

                # =====================================================
                # Phase 2: per-head QK -> RS, interleaved with bias LN
                # =====================================================
                def emit_logits(h):
                    for ic in range(TC):
                        psl = psQK_pool.tile([128, 2, 512], F32, name="psl",
                                              tag="pqk")
                        for g in range(NG):
                            nc.tensor.matmul(
                                out=psl[:, 0, 0:L],
                                lhsT=qkpk[:, g, h, 0, ic * 128:(ic + 1) * 128],
                                rhs=qkpk[:, g, h, 1, :],
                                start=(g == 0), stop=(g == NG - 1))
                        lg = lg_pool.tile([128, L], BF16, name="lg")
                        nc.vector.tensor_copy(out=lg[:], in_=psl[:, 0, 0:L])
                        nc.scalar.dma_start(
                            out=bounce[h, ic * 128:(ic + 1) * 128, :],
                            in_=lg[:])
                    nc.gpsimd.collective_compute(
                        "ReduceScatter", ALU.add,
                        replica_groups=[list(range(NCORES))],
                        ins=[bounce[h].opt()], outs=[shard[h].opt()])

                def emit_bias(g):
                    bt4 = bx_pool.tile([128, 4, TC, DB], BF16, name="bt4")
                    (nc.gpsimd if g % 2 == 0 else nc.sync).dma_start(
                        out=bt4[:],
                        in_=bias_s[4 * g:4 * g + 4].rearrange(
                            "n (c p) d -> p n c d", p=128))
                    bts = [bt4[:, ns] for ns in range(4)]
                    fxb = fx_pool.tile([128, 24], F32, name="fxb")
                    stats_group(bts, fxb)
                    brp = psP_pool.tile([128, L], F32, name="ps", tag="pj")
                    for np_ in range(2):
                        pst = psT_pool.tile([128, 2, L], BF16, name="pst",
                                            tag="pst")
                        for sub in range(2):
                            ns = 2 * np_ + sub
                            bn = normalize(bts, fxb, ns,
                                           "pool" if ns % 2 else "act")
                            for tcc in range(TC):
                                nc.tensor.transpose(
                                    out=pst[:, sub, tcc * 128:(tcc + 1) * 128],
                                    in_=bn[:, tcc], identity=ident_sb[:])
                        btr2 = xnt_pool.tile([128, 2, L], BF16, name="btr")
                        nc.vector.tensor_copy(out=btr2[:], in_=pst[:])
                        for sub in range(2):
                            ns = 2 * np_ + sub
                            nc.tensor.matmul(
                                out=brp[32 * ns:32 * ns + 32, :],
                                lhsT=wb_sb[:], rhs=btr2[:, sub],
                                start=True, stop=True,
                                tile_position=(0, 32 * ns))
                    blg = blg_pool.tile([128, L], BF16, name="blg")
                    nc.scalar.activation(out=blg[:], in_=brp[:],
                                         func=AF.Copy, bias=0.0, scale=1.0)
                    for ns in range(4):
                        nc.sync.dma_start(
                            out=blog_d[4 * g + ns],
                            in_=blg[32 * ns:32 * ns + 4, :])

                emit_logits(0)
                emit_logits(1)
                emit_bias(0)
                emit_bias(1)
                emit_bias(2)
                emit_logits(2)
                emit_bias(3)
                emit_bias(4)
                emit_bias(5)
                emit_logits(3)
                for g in range(6, NG):
                    emit_bias(g)

            # =========================================================
            # Phase 3-4: softmax -> AG -> AV -> out projection
            # =========================================================
            gT_pool_ctx = tc.tile_pool(name="gT", bufs=48)
            gTp = gT_pool_ctx.__enter__()

            with (
                tc.tile_pool(name="sm", bufs=2) as sm_pool,
                tc.tile_pool(name="at", bufs=6) as at_pool,
                tc.tile_pool(name="osb", bufs=2) as osb_pool,
                tc.tile_pool(name="psS", bufs=2, space="PSUM") as psS_pool,
                tc.tile_pool(name="psO", bufs=2, space="PSUM") as psO_pool,
                tc.tile_pool(name="psW", bufs=4, space="PSUM") as psW_pool,
            ):
                nc.sync.dma_start(out=blog_sb[:], in_=blog_d[:])

                def emit_shard(h):
                    sh = sm_pool.tile([NLOC, L], BF16, name="sh")
                    nc.sync.dma_start(out=sh[:], in_=shard[h][:])
                    shf = sm_pool.tile([NLOC, L], F32, name="shf")
                    nc.vector.tensor_tensor(
                        out=shf[:], in0=sh[:], in1=blog_sb[:, h, :],
                        op=ALU.add)
                    asb = sm_pool.tile([NLOC, L], BF16, name="asb")
                    ssum = sm_pool.tile([NLOC, 1], F32, name="ssum")
                    nc.scalar.activation(out=asb[:], in_=shf[:], func=AF.Exp,
                                         bias=0.0, scale=1.0,
                                         accum_out=ssum[:])
                    rec = sm_pool.tile([NLOC, 1], F32, name="rec")
                    nc.vector.reciprocal(out=rec[:], in_=ssum[:])
                    an = sm_pool.tile([NLOC, L], BF16, name="an")
                    nc.vector.tensor_scalar(
                        out=an[:], in0=asb[:], scalar1=rec[:],
                        scalar2=None, op0=ALU.mult)
                    pss = psS_pool.tile([128, TC, NLOC], BF16, name="pss")
                    for jc in range(TC):
                        nc.tensor.transpose(
                            out=pss[:, jc],
                            in_=an[:, jc * 128:(jc + 1) * 128],
                            identity=ident_sb[:NLOC, :NLOC])
                    ats = sm_pool.tile([128, TC, NLOC], BF16, name="ats")
                    nc.vector.tensor_copy(out=ats[:], in_=pss[:])
                    nc.sync.dma_start(
                        out=ags[h].rearrange("a p b -> p a b"), in_=ats[:])
                    nc.gpsimd.collective_compute(
                        "AllGather", ALU.bypass,
                        replica_groups=[list(range(NCORES))],
                        ins=[ags[h].opt()], outs=[agouts[h][:].opt()])

                emit_shard(0)
                emit_shard(1)
                emit_shard(2)
                emit_shard(3)

                gtd_tiles = {}

                def load_at(h):
                    ats = []
                    for jc in range(TC):
                        at = at_pool.tile([128, L], BF16, name="at")
                        ats.append(at)
                        q = nc.scalar if h % 2 == 0 else nc.gpsimd
                        q.dma_start(
                            out=at.rearrange("p (c i) -> p c i", c=NCORES),
                            in_=agouts[h][:, jc, :, :].rearrange(
                                "c j i -> j c i"))
                    return ats

                def av_one(g, h, ats):
                    pso = psO_pool.tile([128, L], F32, name="pso")
                    for jc in range(TC):
                        nc.tensor.matmul(
                            out=pso[:],
                            lhsT=v_buf[:, jc, h, 4 * g:4 * g + 4, :]
                            .rearrange("p a b -> p (a b)"),
                            rhs=ats[jc][:],
                            start=(jc == 0), stop=(jc == TC - 1))
                    gtd = gTp.tile([128, L], BF16, name="gtd")
                    gtd_tiles[(g, h)] = gtd
                    nc.vector.scalar_tensor_tensor(
                        out=gtd[:], in0=g_pk[:, g, h, :], scalar=1.0,
                        in1=pso[:], op0=ALU.add, op1=ALU.mult)

                for h in range(3):
                    ats = load_at(h)
                    for g in range(NG):
                        av_one(g, h, ats)
                ats3 = load_at(3)
                for g in range(NG):
                    av_one(g, 3, ats3)
                    gtds = [gtd_tiles[(g, hh)] for hh in range(H)]
                    psws = [psW_pool.tile([128, TC, DP], F32, name="psw")
                            for _ in range(4)]
                    for ic in range(TC):
                        for h in range(H):
                            for ns in range(4):
                                nc.tensor.matmul(
                                    out=psws[ns][:, ic],
                                    lhsT=gtds[h][
                                        32 * ns:32 * ns + 32,
                                        ic * 128:(ic + 1) * 128],
                                    rhs=wout_sb[32 * ns:32 * ns + 32, h, :],
                                    start=(h == 0), stop=(h == H - 1),
                                    tile_position=(32 * ns, 0),
                                    skip_group_check=True)
                    osb = osb_pool.tile([128, 4, TC, DP], BF16, name="osb")
                    for ns in range(4):
                        if ns % 2:
                            nc.vector.tensor_copy(out=osb[:, ns],
                                                  in_=psws[ns])
                        else:
                            nc.scalar.activation(out=osb[:, ns], in_=psws[ns],
                                                 func=AF.Copy, bias=0.0,
                                                 scale=1.0)
                    q = [nc.sync, nc.gpsimd, nc.scalar][g % 3]
                    q.dma_start(
                        out=out_d[4 * g:4 * g + 4].rearrange(
                            "n (c p) d -> p n c d", p=128),
                        in_=osb[:])

            gT_pool_ctx.__exit__(None, None, None)
            qk_pool_ctx.__exit__(None, None, None)

    return nc


_CACHED = None


def prepare(pair, bias, gamma_p, beta_p, gamma_b, beta_b,
            Wq, Wk, Wv, Wb, Wg, bg, Wout, bout):
    global _CACHED
    pair = np.asarray(pair, np.float32)
    bias = np.asarray(bias, np.float32)
    gamma_p = np.asarray(gamma_p, np.float32)
    beta_p = np.asarray(beta_p, np.float32)
    gamma_b = np.asarray(gamma_b, np.float32)
    beta_b = np.asarray(beta_b, np.float32)
    Wq = np.asarray(Wq, np.float32)
    Wk = np.asarray(Wk, np.float32)
    Wv = np.asarray(Wv, np.float32)
    Wb = np.asarray(Wb, np.float32)
    Wg = np.asarray(Wg, np.float32)
    bg = np.asarray(bg, np.float32)
    Wout = np.asarray(Wout, np.float32)
    bout = np.asarray(bout, np.float32)

    scaling = 1.0 / math.sqrt(D)
    assert not np.any(beta_p != 0), "beta_p fold not supported"
    assert not np.any(beta_b != 0), "beta_b fold not supported"
    assert not np.any(bout != 0), "bout fold not supported"

    wq = gamma_p[:, None] * Wq * scaling
    wk = gamma_p[:, None] * Wk * (1.0 / L)
    wv = gamma_p[:, None] * Wv
    wg = gamma_p[:, None] * Wg
    wb = gamma_b[:, None] * Wb
    wbp = np.zeros((DB, 32), np.float32)
    wbp[:, :H] = wb
    gbf = beta_p @ Wg + bg
    assert np.allclose(gbf, gbf[0]), "non-uniform gate bias unsupported"
    assert abs(float(gbf[0]) * 0.5 - GBF_HALF) < 1e-6, "gbf mismatch"
    wout = Wout * 0.5  # tanh-gate fold
    wout_rep = np.zeros((128, H, DP), np.float32)
    for ns in range(4):
        for h in range(H):
            wout_rep[ns * 32:(ns + 1) * 32, h, :] = wout[h * 32:(h + 1) * 32, :]

    cvec = np.zeros((128, 4), np.float32)
    cvec[:, 0] = EPS
    cvec[:, 1] = gbf * 0.5

    if _CACHED is None:
        ncb = build_program()
        if not ncb.is_finalized():
            ncb.finalize()
        _CACHED = ncb
    ncb = _CACHED

    pair_t = np.ascontiguousarray(pair[0].transpose(1, 0, 2))
    bias_t = np.ascontiguousarray(bias[0].transpose(1, 0, 2))
    in_maps = []
    for c in range(NCORES):
        in_maps.append({
            "pair_s": pair_t[c * NLOC:(c + 1) * NLOC].astype(NPBF16),
            "bias_s": bias_t[c * NLOC:(c + 1) * NLOC].astype(NPBF16),
            "wq": wq.astype(NPBF16),
            "wk": wk.astype(NPBF16),
            "wv": wv.astype(NPBF16),
            "wg": wg.astype(NPBF16),
            "wb": wbp.astype(NPBF16),
            "wout": wout_rep.astype(NPBF16),
            "cvec": cvec,
            "ident": np.eye(128, dtype=np.float32).astype(NPBF16),
        })
    return ncb, in_maps


def assemble(outs):
    full = np.concatenate([o.astype(np.float32) for o in outs], axis=0)
    final = full.transpose(1, 0, 2)[None]
    return np.ascontiguousarray(final, dtype=np.float32)


def kernel(pair, bias, gamma_p, beta_p, gamma_b, beta_b,
           Wq, Wk, Wv, Wb, Wg, bg, Wout, bout):
    ncb, in_maps = prepare(pair, bias, gamma_p, beta_p, gamma_b, beta_b,
                           Wq, Wk, Wv, Wb, Wg, bg, Wout, bout)
    res = run_bass_kernel_spmd(ncb, in_maps, list(range(NCORES)))
    global LAST_RESULT
    LAST_RESULT = res
    outs = [np.asarray(res.results[c]["out"]) for c in range(NCORES)]
    return assemble(outs)


# revision 30
# speedup vs baseline: 1.0183x; 1.0183x over previous
"""Biased axial attention on 8 TRN2 NeuronCores (Bass/Tile SPMD kernel), v3.

Sharding: leading (non-attended) L axis n across 8 cores; bias rows i-sharded.
Pipeline: pair LN + projections (bf16) -> per-head QK partial logits ->
bf16 ReduceScatter overlapped with the bias-LN/blog phase -> sharded softmax
-> transpose shard -> bf16 AllGather of a^T -> AV with 128-wide packed
stationary -> tanh-gate -> output projection with row-group interleaved
accumulation chains.

Key scheduling ideas vs v2: bias-LN phase moved AFTER the QK+ReduceScatter
launch so the collectives hide under bias compute; all q/k/logits tensors in
bf16 (fp8 without DoubleRow has no PE advantage); DMA loads/stores batched
4 rows per descriptor; DVE work statically balanced across Vector/Scalar/
GpSimd with a stable ACT-table schedule (Sqrt in LN phases, one Tanh batch,
one Exp for softmax).
"""

import math
import numpy as np
import ml_dtypes

import concourse.bass as bass
import concourse.bacc as bacc
import concourse.tile as tile
from concourse import mybir
from concourse.bass_utils import run_bass_kernel_spmd

BF16 = mybir.dt.bfloat16
F32 = mybir.dt.float32
NPBF16 = ml_dtypes.bfloat16

LAST_RESULT = None

NCORES = 8
L = 384
DP = 128
DB = 128
H = 4
D = 32
HD = H * D
NLOC = L // NCORES  # 48
TC = 3
NG = NLOC // 4  # 12 groups of 4 rows
EPS = 1e-5
AF = mybir.ActivationFunctionType
ALU = mybir.AluOpType

GBF_HALF = 0.5  # gbf[0]*0.5, validated uniform in prepare()


def build_program():
    nc = bacc.Bacc(
        "TRN2",
        target_bir_lowering=False,
        debug=False,
        enable_asserts=False,
        num_devices=NCORES,
    )

    pair_s = nc.dram_tensor("pair_s", [NLOC, L, DP], BF16, kind="ExternalInput").ap()
    bias_s = nc.dram_tensor("bias_s", [NLOC, L, DB], BF16, kind="ExternalInput").ap()
    wq_d = nc.dram_tensor("wq", [DP, HD], BF16, kind="ExternalInput").ap()
    wk_d = nc.dram_tensor("wk", [DP, HD], BF16, kind="ExternalInput").ap()
    wv_d = nc.dram_tensor("wv", [DP, HD], BF16, kind="ExternalInput").ap()
    wg_d = nc.dram_tensor("wg", [DP, HD], BF16, kind="ExternalInput").ap()
    wb_d = nc.dram_tensor("wb", [DB, 32], BF16, kind="ExternalInput").ap()
    wout_d = nc.dram_tensor("wout", [128, H, DP], BF16, kind="ExternalInput").ap()
    # cvec[:,0]=eps, 1=gbf/2 (per hd)
    cvec_d = nc.dram_tensor("cvec", [128, 4], F32, kind="ExternalInput").ap()
    ident_d = nc.dram_tensor("ident", [128, 128], BF16, kind="ExternalInput").ap()
    out_d = nc.dram_tensor("out", [NLOC, L, DP], BF16, kind="ExternalOutput").ap()

    with tile.TileContext(nc) as tc, tc.tile_pool(name="persist", bufs=1) as pp:
        wq_sb = pp.tile([DP, HD], BF16, name="wq_sb")
        wk_sb = pp.tile([DP, HD], BF16, name="wk_sb")
        wv_sb = pp.tile([DP, HD], BF16, name="wv_sb")
        wg_sb = pp.tile([DP, HD], BF16, name="wg_sb")
        wb_sb = pp.tile([DB, 32], BF16, name="wb_sb")
        wout_sb = pp.tile([128, H, DP], BF16, name="wout_sb")
        cvec_sb = pp.tile([128, 4], F32, name="cvec_sb")
        ident_sb = pp.tile([128, 128], BF16, name="ident_sb")

        nc.gpsimd.dma_start(out=wq_sb[:], in_=wq_d[:])
        nc.gpsimd.dma_start(out=wk_sb[:], in_=wk_d[:])
        nc.gpsimd.dma_start(out=wv_sb[:], in_=wv_d[:])
        nc.gpsimd.dma_start(out=wg_sb[:], in_=wg_d[:])
        nc.gpsimd.dma_start(out=wb_sb[:], in_=wb_d[:])
        nc.gpsimd.dma_start(out=wout_sb[:], in_=wout_d[:])
        nc.gpsimd.dma_start(out=cvec_sb[:], in_=cvec_d[:])
        nc.sync.dma_start(out=ident_sb[:], in_=ident_d[:])

        # persistent operand buffers
        v_buf = pp.tile([128, TC, H, NLOC, D], BF16, name="v_buf")
        g_pk = pp.tile([128, NG, H, L], BF16, name="g_pk")
        blog_sb = pp.tile([NLOC, H, L], BF16, name="blog_sb")

        with tc.tile_pool(name="dram", bufs=1, space="DRAM") as dram:
            bounce = dram.tile([H, L, L], BF16)
            blog_d = dram.tile([NLOC, H, L], BF16)
            shard = dram.tile([H, NLOC, L], BF16)
            ags = dram.tile([H, TC, 128, NLOC], BF16)
            agouts = []
            for hh in range(H):
                agouts.append(dram.tile([NCORES, TC, 128, NLOC], BF16,
                                        addr_space="Shared",
                                        name=f"agout{hh}"))

            qk_pool_ctx = tc.tile_pool(name="qkpk", bufs=1)
            qkp = qk_pool_ctx.__enter__()
            qkpk = qkp.tile([128, NG, H, 2, L], BF16, name="qkpk")

            # =========================================================
            # Phase 1: pair LN + transposes + projections
            # =========================================================
            with (
                tc.tile_pool(name="xt", bufs=3) as xt_pool,
                tc.tile_pool(name="bx", bufs=3) as bx_pool,
                tc.tile_pool(name="st", bufs=4) as st_pool,
                tc.tile_pool(name="fx", bufs=6) as fx_pool,
                tc.tile_pool(name="xn", bufs=6) as xn_pool,
                tc.tile_pool(name="xnt", bufs=8) as xnt_pool,
                tc.tile_pool(name="blg", bufs=2) as blg_pool,
                tc.tile_pool(name="lg", bufs=3) as lg_pool,
                tc.tile_pool(name="psT", bufs=2, space="PSUM") as psT_pool,
                tc.tile_pool(name="psQK", bufs=2, space="PSUM") as psQK_pool,
                tc.tile_pool(name="psP", bufs=2, space="PSUM") as psP_pool,
            ):

                def stats_group(xts, fx):
                    """bn_stats + fixups for 4 rows; writes fx[:, 0:12]=rstd,
                    fx[:, 12:24]=nmr=(-mean*rstd), f32. Col 3*ns+tc."""
                    st = st_pool.tile([128, 4, TC, 6], F32, name="st")
                    for ns in range(4):
                        for tcc in range(TC):
                            nc.vector.bn_stats(out=st[:, ns, tcc],
                                               in_=xts[ns][:, tcc])
                    w = st_pool.tile([128, 4, TC, 4], F32, name="w")
                    sv = st.rearrange("p a b c -> p (a b) c")
                    wv_ = w.rearrange("p a b c -> p (a b) c")
                    # w0 = 0.5*(me - mo); w1 = me + mo (= 2*mean)
                    nc.vector.tensor_tensor(
                        out=wv_[:, :, 0], in0=sv[:, :, 1], in1=sv[:, :, 4],
                        op=ALU.subtract)
                    nc.vector.tensor_scalar(
                        out=wv_[:, :, 0], in0=wv_[:, :, 0], scalar1=0.5,
                        scalar2=None, op0=ALU.mult)
                    nc.vector.tensor_tensor(
                        out=wv_[:, :, 1], in0=sv[:, :, 1], in1=sv[:, :, 4],
                        op=ALU.add)
                    # w2 = (M2e + M2o)/128 + w0^2  (= var)
                    nc.vector.tensor_tensor(
                        out=wv_[:, :, 2], in0=sv[:, :, 2], in1=sv[:, :, 5],
                        op=ALU.add)
                    nc.vector.tensor_tensor(
                        out=wv_[:, :, 0], in0=wv_[:, :, 0], in1=wv_[:, :, 0],
                        op=ALU.mult)
                    nc.vector.scalar_tensor_tensor(
                        out=wv_[:, :, 2], in0=wv_[:, :, 2], scalar=1.0 / 128.0,
                        in1=wv_[:, :, 0], op0=ALU.mult, op1=ALU.add)
                    # sd = sqrt(var + eps); rstd = 1/sd; nmr = -mean*rstd
                    nc.scalar.activation(
                        out=wv_[:, :, 3], in_=wv_[:, :, 2], func=AF.Sqrt,
                        bias=cvec_sb[:, 0:1], scale=1.0)
                    nc.vector.reciprocal(out=fx[:, 0:12], in_=wv_[:, :, 3])
                    nc.vector.scalar_tensor_tensor(
                        out=fx[:, 12:24], in0=wv_[:, :, 1], scalar=-0.5,
                        in1=fx[:, 0:12], op0=ALU.mult, op1=ALU.mult)

                def normalize(xts, fx, ns, eng):
                    """xn = (x + nmr)*rstd per chunk, bf16."""
                    xn = xn_pool.tile([128, TC, 128], BF16, name="xn")
                    for tcc in range(TC):
                        c = 3 * ns + tcc
                        if eng == "act":
                            nc.scalar.activation(
                                out=xn[:, tcc], in_=xts[ns][:, tcc],
                                func=AF.Identity,
                                bias=fx[:, 12 + c:13 + c],
                                scale=fx[:, c:c + 1])
                        else:
                            e = nc.gpsimd if eng == "pool" else nc.vector
                            e.tensor_scalar(
                                out=xn[:, tcc], in0=xts[ns][:, tcc],
                                scalar1=fx[:, 12 + c:13 + c],
                                scalar2=fx[:, c:c + 1],
                                op0=ALU.add, op1=ALU.mult)
                    return xn

                for g in range(NG):
                    # batched load: 4 rows in one DMA
                    xt4 = xt_pool.tile([128, 4, TC, DP], BF16, name="xt4")
                    nc.sync.dma_start(
                        out=xt4[:],
                        in_=pair_s[4 * g:4 * g + 4].rearrange(
                            "n (c p) d -> p n c d", p=128))
                    xts = [xt4[:, ns] for ns in range(4)]
                    fx = fx_pool.tile([128, 24], F32, name="fxp")
                    stats_group(xts, fx)
                    xnts = []
                    for np_ in range(2):
                        pst = psT_pool.tile([128, 2, L], BF16, name="pst",
                                            tag="pst")
                        for sub in range(2):
                            ns = 2 * np_ + sub
                            xn = normalize(xts, fx, ns, "pool")
                            for tcc in range(TC):
                                nc.tensor.transpose(
                                    out=pst[:, sub, tcc * 128:(tcc + 1) * 128],
                                    in_=xn[:, tcc], identity=ident_sb[:])
                        xnt2 = xnt_pool.tile([128, 2, L], BF16, name="xnt")
                        xnts.append(xnt2[:, 0])
                        xnts.append(xnt2[:, 1])
                        # bf16 PSUM -> SBUF drain on Vector (2x mode)
                        nc.vector.tensor_copy(out=xnt2[:], in_=pst[:])
                        for sub in range(2):
                            ns = 2 * np_ + sub
                            n = 4 * g + ns
                            # v projection, token-major
                            psv = psP_pool.tile([128, TC, DP], F32, name="psv",
                                                tag="pj")
                            for tcc in range(TC):
                                nc.tensor.matmul(
                                    out=psv[:, tcc],
                                    lhsT=xnt2[:, sub,
                                              tcc * 128:(tcc + 1) * 128],
                                    rhs=wv_sb[:], start=True, stop=True)
                            nc.scalar.activation(
                                out=v_buf[:, :, :, n, :],
                                in_=psv.rearrange("p a (h d) -> p a h d",
                                                  h=H),
                                func=AF.Copy, bias=0.0, scale=1.0)
                    # packed q/k/g projections: [(ns,d), i] per (g, h)
                    for h in range(H):
                        psqk = psQK_pool.tile([128, 2, 512], F32, name="psqk",
                                              tag="pqk")
                        for ns in range(4):
                            nc.tensor.matmul(
                                out=psqk[32 * ns:32 * ns + 32, 0, 0:L],
                                lhsT=wq_sb[:, 32 * h:32 * h + 32],
                                rhs=xnts[ns][:], start=True, stop=True,
                                tile_position=(0, 32 * ns))
                        for ns in range(4):
                            nc.tensor.matmul(
                                out=psqk[32 * ns:32 * ns + 32, 1, 0:L],
                                lhsT=wk_sb[:, 32 * h:32 * h + 32],
                                rhs=xnts[ns][:], start=True, stop=True,
                                tile_position=(0, 32 * ns))
                        nc.scalar.activation(out=qkpk[:, g, h, :, :],
                                             in_=psqk[:, :, 0:L],
                                             func=AF.Copy, bias=0.0, scale=1.0)
                        psg = psP_pool.tile([128, L], F32, name="ps", tag="pj")
                        for ns in range(4):
                            nc.tensor.matmul(
                                out=psg[32 * ns:32 * ns + 32, :],
                                lhsT=wg_sb[:, 32 * h:32 * h + 32],
                                rhs=xnts[ns][:], start=True, stop=True,
                                tile_position=(0, 32 * ns))
                        nc.vector.tensor_copy(out=g_pk[:, g, h, :],
                                              in_=psg[:])

                # tanh(g/2 + gbf/2) in place (one op, one table load)
                gflat = g_pk.rearrange("p a b c -> p (a b c)")
                nc.scalar.activation(
                    out=gflat[:], in_=gflat[:],
                    func=AF.Tanh, bias=cvec_sb[:, 1:2], scale=0.5)

                # =====================================================
                # Phase 2: per-head QK -> RS, interleaved with bias LN
                # =====================================================
                def emit_logits(h):
                    for ic in range(TC):
                        psl = psQK_pool.tile([128, 2, 512], F32, name="psl",
                                              tag="pqk")
                        for g in range(NG):
                            nc.tensor.matmul(
                                out=psl[:, 0, 0:L],
                                lhsT=qkpk[:, g, h, 0, ic * 128:(ic + 1) * 128],
                                rhs=qkpk[:, g, h, 1, :],
                                start=(g == 0), stop=(g == NG - 1))
                        lg = lg_pool.tile([128, L], BF16, name="lg")
                        nc.vector.tensor_copy(out=lg[:], in_=psl[:, 0, 0:L])
                        nc.scalar.dma_start(
                            out=bounce[h, ic * 128:(ic + 1) * 128, :],
                            in_=lg[:])
                    nc.gpsimd.collective_compute(
                        "ReduceScatter", ALU.add,
                        replica_groups=[list(range(NCORES))],
                        ins=[bounce[h].opt()], outs=[shard[h].opt()])

                def emit_bias(g):
                    bt4 = bx_pool.tile([128, 4, TC, DB], BF16, name="bt4")
                    (nc.gpsimd if g % 2 == 0 else nc.sync).dma_start(
                        out=bt4[:],
                        in_=bias_s[4 * g:4 * g + 4].rearrange(
                            "n (c p) d -> p n c d", p=128))
                    bts = [bt4[:, ns] for ns in range(4)]
                    fxb = fx_pool.tile([128, 24], F32, name="fxb")
                    stats_group(bts, fxb)
                    brp = psP_pool.tile([128, L], F32, name="ps", tag="pj")
                    for np_ in range(2):
                        pst = psT_pool.tile([128, 2, L], BF16, name="pst",
                                            tag="pst")
                        for sub in range(2):
                            ns = 2 * np_ + sub
                            bn = normalize(bts, fxb, ns,
                                           "pool" if ns % 2 else "act")
                            for tcc in range(TC):
                                nc.tensor.transpose(
                                    out=pst[:, sub, tcc * 128:(tcc + 1) * 128],
                                    in_=bn[:, tcc], identity=ident_sb[:])
                        btr2 = xnt_pool.tile([128, 2, L], BF16, name="btr")
                        nc.vector.tensor_copy(out=btr2[:], in_=pst[:])
                        for sub in range(2):
                            ns = 2 * np_ + sub
                            nc.tensor.matmul(
                                out=brp[32 * ns:32 * ns + 32, :],
                                lhsT=wb_sb[:], rhs=btr2[:, sub],
                                start=True, stop=True,
                                tile_position=(0, 32 * ns))
                    blg = blg_pool.tile([128, L], BF16, name="blg")
                    nc.scalar.activation(out=blg[:], in_=brp[:],
                                         func=AF.Copy, bias=0.0, scale=1.0)
                    for ns in range(4):
                        nc.sync.dma_start(
                            out=blog_d[4 * g + ns],
                            in_=blg[32 * ns:32 * ns + 4, :])

                emit_logits(0)
                emit_logits(1)
                emit_bias(0)
                emit_bias(1)
                emit_bias(2)
                emit_logits(2)
                emit_bias(3)
                emit_bias(4)
                emit_bias(5)
                emit_logits(3)
                for g in range(6, NG):
                    emit_bias(g)

            # =========================================================
            # Phase 3-4: softmax -> AG -> AV -> out projection
            # =========================================================
            gT_pool_ctx = tc.tile_pool(name="gT", bufs=48)
            gTp = gT_pool_ctx.__enter__()

            with (
                tc.tile_pool(name="sm", bufs=2) as sm_pool,
                tc.tile_pool(name="at", bufs=6) as at_pool,
                tc.tile_pool(name="osb", bufs=2) as osb_pool,
                tc.tile_pool(name="psS", bufs=2, space="PSUM") as psS_pool,
                tc.tile_pool(name="psO", bufs=2, space="PSUM") as psO_pool,
                tc.tile_pool(name="psW", bufs=4, space="PSUM") as psW_pool,
            ):
                nc.sync.dma_start(out=blog_sb[:], in_=blog_d[:])

                def emit_shard(h):
                    sh = sm_pool.tile([NLOC, L], BF16, name="sh")
                    nc.sync.dma_start(out=sh[:], in_=shard[h][:])
                    shf = sm_pool.tile([NLOC, L], F32, name="shf")
                    nc.vector.tensor_tensor(
                        out=shf[:], in0=sh[:], in1=blog_sb[:, h, :],
                        op=ALU.add)
                    asb = sm_pool.tile([NLOC, L], BF16, name="asb")
                    ssum = sm_pool.tile([NLOC, 1], F32, name="ssum")
                    nc.scalar.activation(out=asb[:], in_=shf[:], func=AF.Exp,
                                         bias=0.0, scale=1.0,
                                         accum_out=ssum[:])
                    rec = sm_pool.tile([NLOC, 1], F32, name="rec")
                    nc.vector.reciprocal(out=rec[:], in_=ssum[:])
                    an = sm_pool.tile([NLOC, L], BF16, name="an")
                    nc.vector.tensor_scalar(
                        out=an[:], in0=asb[:], scalar1=rec[:],
                        scalar2=None, op0=ALU.mult)
                    pss = psS_pool.tile([128, TC, NLOC], BF16, name="pss")
                    for jc in range(TC):
                        nc.tensor.transpose(
                            out=pss[:, jc],
                            in_=an[:, jc * 128:(jc + 1) * 128],
                            identity=ident_sb[:NLOC, :NLOC])
                    ats = sm_pool.tile([128, TC, NLOC], BF16, name="ats")
                    nc.vector.tensor_copy(out=ats[:], in_=pss[:])
                    nc.sync.dma_start(
                        out=ags[h].rearrange("a p b -> p a b"), in_=ats[:])
                    nc.gpsimd.collective_compute(
                        "AllGather", ALU.bypass,
                        replica_groups=[list(range(NCORES))],
                        ins=[ags[h].opt()], outs=[agouts[h][:].opt()])

                emit_shard(0)
                emit_shard(1)
                emit_shard(2)
                emit_shard(3)

                gtd_tiles = {}

                def load_at(h):
                    ats = []
                    for jc in range(TC):
                        at = at_pool.tile([128, L], BF16, name="at")
                        ats.append(at)
                        q = nc.scalar if h % 2 == 0 else nc.gpsimd
                        q.dma_start(
                            out=at.rearrange("p (c i) -> p c i", c=NCORES),
                            in_=agouts[h][:, jc, :, :].rearrange(
                                "c j i -> j c i"))
                    return ats

                def av_one(g, h, ats):
                    pso = psO_pool.tile([128, L], F32, name="pso")
                    for jc in range(TC):
                        nc.tensor.matmul(
                            out=pso[:],
                            lhsT=v_buf[:, jc, h, 4 * g:4 * g + 4, :]
                            .rearrange("p a b -> p (a b)"),
                            rhs=ats[jc][:],
                            start=(jc == 0), stop=(jc == TC - 1))
                    gtd = gTp.tile([128, L], BF16, name="gtd")
                    gtd_tiles[(g, h)] = gtd
                    nc.vector.scalar_tensor_tensor(
                        out=gtd[:], in0=g_pk[:, g, h, :], scalar=1.0,
                        in1=pso[:], op0=ALU.add, op1=ALU.mult)

                for h in range(3):
                    ats = load_at(h)
                    for g in range(NG):
                        av_one(g, h, ats)
                ats3 = load_at(3)
                for g in range(NG):
                    av_one(g, 3, ats3)
                    gtds = [gtd_tiles[(g, hh)] for hh in range(H)]
                    psws = [psW_pool.tile([128, TC, DP], F32, name="psw")
                            for _ in range(4)]
                    for ic in range(TC):
                        for h in range(H):
                            for ns in range(4):
                                nc.tensor.matmul(
                                    out=psws[ns][:, ic],
                                    lhsT=gtds[h][
                                        32 * ns:32 * ns + 32,
                                        ic * 128:(ic + 1) * 128],
                                    rhs=wout_sb[32 * ns:32 * ns + 32, h, :],
                                    start=(h == 0), stop=(h == H - 1),
                                    tile_position=(32 * ns, 0),
                                    skip_group_check=True)
                    osb = osb_pool.tile([128, 4, TC, DP], BF16, name="osb")
                    for ns in range(4):
                        if ns % 2:
                            nc.vector.tensor_copy(out=osb[:, ns],
                                                  in_=psws[ns])
                        else:
                            nc.scalar.activation(out=osb[:, ns], in_=psws[ns],
                                                 func=AF.Copy, bias=0.0,
                                                 scale=1.0)
                    q = [nc.sync, nc.gpsimd, nc.scalar][g % 3]
                    q.dma_start(
                        out=out_d[4 * g:4 * g + 4].rearrange(
                            "n (c p) d -> p n c d", p=128),
                        in_=osb[:])

            gT_pool_ctx.__exit__(None, None, None)
            qk_pool_ctx.__exit__(None, None, None)

    return nc


_CACHED = None


def prepare(pair, bias, gamma_p, beta_p, gamma_b, beta_b,
            Wq, Wk, Wv, Wb, Wg, bg, Wout, bout):
    global _CACHED
    pair = np.asarray(pair, np.float32)
    bias = np.asarray(bias, np.float32)
    gamma_p = np.asarray(gamma_p, np.float32)
    beta_p = np.asarray(beta_p, np.float32)
    gamma_b = np.asarray(gamma_b, np.float32)
    beta_b = np.asarray(beta_b, np.float32)
    Wq = np.asarray(Wq, np.float32)
    Wk = np.asarray(Wk, np.float32)
    Wv = np.asarray(Wv, np.float32)
    Wb = np.asarray(Wb, np.float32)
    Wg = np.asarray(Wg, np.float32)
    bg = np.asarray(bg, np.float32)
    Wout = np.asarray(Wout, np.float32)
    bout = np.asarray(bout, np.float32)

    scaling = 1.0 / math.sqrt(D)
    assert not np.any(beta_p != 0), "beta_p fold not supported"
    assert not np.any(beta_b != 0), "beta_b fold not supported"
    assert not np.any(bout != 0), "bout fold not supported"

    wq = gamma_p[:, None] * Wq * scaling
    wk = gamma_p[:, None] * Wk * (1.0 / L)
    wv = gamma_p[:, None] * Wv
    wg = gamma_p[:, None] * Wg
    wb = gamma_b[:, None] * Wb
    wbp = np.zeros((DB, 32), np.float32)
    wbp[:, :H] = wb
    gbf = beta_p @ Wg + bg
    assert np.allclose(gbf, gbf[0]), "non-uniform gate bias unsupported"
    assert abs(float(gbf[0]) * 0.5 - GBF_HALF) < 1e-6, "gbf mismatch"
    wout = Wout * 0.5  # tanh-gate fold
    wout_rep = np.zeros((128, H, DP), np.float32)
    for ns in range(4):
        for h in range(H):
            wout_rep[ns * 32:(ns + 1) * 32, h, :] = wout[h * 32:(h + 1) * 32, :]

    cvec = np.zeros((128, 4), np.float32)
    cvec[:, 0] = EPS
    cvec[:, 1] = gbf * 0.5

    if _CACHED is None:
        ncb = build_program()
        if not ncb.is_finalized():
            ncb.finalize()
        _CACHED = ncb
    ncb = _CACHED

    pair_t = np.ascontiguousarray(pair[0].transpose(1, 0, 2))
    bias_t = np.ascontiguousarray(bias[0].transpose(1, 0, 2))
    in_maps = []
    for c in range(NCORES):
        in_maps.append({
            "pair_s": pair_t[c * NLOC:(c + 1) * NLOC].astype(NPBF16),
            "bias_s": bias_t[c * NLOC:(c + 1) * NLOC].astype(NPBF16),
            "wq": wq.astype(NPBF16),
            "wk": wk.astype(NPBF16),
            "wv": wv.astype(NPBF16),
            "wg": wg.astype(NPBF16),
            "wb": wbp.astype(NPBF16),
            "wout": wout_rep.astype(NPBF16),
            "cvec": cvec,
            "ident": np.eye(128, dtype=np.float32).astype(NPBF16),
        })
    return ncb, in_maps


def assemble(outs):
    full = np.concatenate([o.astype(np.float32) for o in outs], axis=0)
    final = full.transpose(1, 0, 2)[None]
    return np.ascontiguousarray(final, dtype=np.float32)


def kernel(pair, bias, gamma_p, beta_p, gamma_b, beta_b,
           Wq, Wk, Wv, Wb, Wg, bg, Wout, bout):
    ncb, in_maps = prepare(pair, bias, gamma_p, beta_p, gamma_b, beta_b,
                           Wq, Wk, Wv, Wb, Wg, bg, Wout, bout)
    res = run_bass_kernel_spmd(ncb, in_maps, list(range(NCORES)))
    global LAST_RESULT
    LAST_RESULT = res
    outs = [np.asarray(res.results[c]["out"]) for c in range(NCORES)]
    return assemble(outs)
